# revision 34
# baseline (speedup 1.0000x reference)
# Trainium2 Bass kernel for NonLocalBlock (B=4, C=64, CI=32, H=W=80).
#
# Math (per batch, N = H*W = 6400):
#   u = Wu@x+bu, v = Wv@x+bv, g = Wg@x+bg           [CI, N]
#   f[n,m] = sum_c u[c,n] v[c,m]; softmax over n (axis=1 of f)
#   y[c,n] = sum_m f_sm[n,m] g[c,m];  out = Ww@y + bw + x
#
# Define S = v^T u  (S[m,n] = f[n,m]).  The softmax axis n is the FREE
# axis of S rows, so 128-row blocks of S make the softmax row-local.
# y = g @ softmax_rows(S).
#
# Sharding: 8 cores = 4 batches x 2 halves of the m axis.  Each core
# computes a partial y over its 3200 m rows and applies the output
# projection; the host sums the halves and adds bias + residual.  The
# odd core's x_aug is column-rotated by 3200 so the same SPMD program
# reads its v/g region at columns 0-3200; the host un-rotates its
# output (a pure relabeling of the n axis — softmax rows are invariant).
#
# Engine budget per core: ACT does only the 175 exp ACTIVATEs
# (~7.2us/block, the bottleneck).  Row sums come from a DVE pairwise
# bf16 add tree (2x_1p mode) + one small 1x reduce (~4.4us/block);
# ACT-side accum_out costs a serial READ_ACCUMULATOR per chunk and
# DVE-side tensor_scalar+accum lowers to the 1x CACHE_REDUCE — both
# measurably slower.  All PSUM->SBUF copies live off the ACT queue.
import numpy as np

import concourse.bass as bass
import concourse.mybir as mybir
from concourse import bacc, tile
from concourse.bass_utils import run_bass_kernel_spmd

F32 = mybir.dt.float32
BF16 = mybir.dt.bfloat16
F16 = mybir.dt.float16

B, C, CI, H, W = 4, 64, 32, 80, 80
N = H * W              # 6400
NCORES = 8
MH = N // 2            # 3200 rows of S per core
MB = 128               # S row-block
NBLK = MH // MB        # 25 blocks per core
SCH = 1024             # S free-dim chunk held in PSUM (2 banks)

EXP = mybir.ActivationFunctionType.Exp
ADD = mybir.AluOpType.add


def _ceil_chunks(total, step):
    out = []
    off = 0
    while off < total:
        out.append((off, min(step, total - off)))
        off += step
    return out


# 256-wide chunk FIRST: the next block's first S matmul then reuses a
# buffer whose last reader is the second-to-last exp of this block, so
# the PE builds it while ACT finishes the block (no boundary stall).
S_CHUNKS = [(0, 256)] + [(256 + SCH * k, SCH) for k in range(6)]
Y_CHUNKS = _ceil_chunks(N, 512)      # 12 x 512 + 256
V_CHUNKS = _ceil_chunks(MH, 512)     # 6 x 512 + 128


def _yslot(j):
    """y chunk j -> (tile index 0/1, partition slot, col half)."""
    if j < 8:
        return 0, j % 4, j // 4
    return 1, (j - 8) % 4, (j - 8) // 4


def build_nc():
    nc = bacc.Bacc("TRN2", target_bir_lowering=False, debug=False,
                   num_devices=NCORES)

    x_aug_d = nc.dram_tensor("x_aug", [C + 1, N], F16, kind="ExternalInput")
    wuT_d = nc.dram_tensor("wuT", [C + 1, CI], F16, kind="ExternalInput")
    wvT_d = nc.dram_tensor("wvT", [C + 1, CI], F16, kind="ExternalInput")
    wgT_d = nc.dram_tensor("wgT", [C + 1, CI], F16, kind="ExternalInput")
    wwT4_d = nc.dram_tensor("wwT4", [128, C], F16, kind="ExternalInput")
    out_d = nc.dram_tensor("out", [C, N], F16, kind="ExternalOutput")

    with tile.TileContext(nc) as tc:
        with (
            tc.tile_pool(name="const", bufs=1) as cpool,
            tc.tile_pool(name="big", bufs=2) as dpool,
            tc.tile_pool(name="small", bufs=3) as wpool,
        ):
            # ---- persistent SBUF tiles ----
            x_aug = cpool.tile([C + 1, N], F16, tag="xa")
            u_sb = cpool.tile([2 * CI, N], F16, tag="u")     # 2 row groups
            v_sb = cpool.tile([2 * CI, MH], F16, tag="v")
            gt_sb = cpool.tile([128, NBLK * CI], F32, tag="gt")
            wuT = cpool.tile([C + 1, CI], F16, tag="wu")
            wvT = cpool.tile([C + 1, CI], F16, tag="wv")
            wgT = cpool.tile([C + 1, CI], F16, tag="wg")
            wwT4 = cpool.tile([128, C], F16, tag="ww")
            ysA = cpool.tile([128, 1024], F16, tag="ysA")
            ysB = cpool.tile([128, 768], F16, tag="ysB")

            # ---- input DMAs: weights first, x_aug striped over queues ----
            nc.gpsimd.dma_start(wuT[:], wuT_d[:])
            nc.gpsimd.dma_start(wvT[:], wvT_d[:])
            nc.gpsimd.dma_start(wgT[:], wgT_d[:])
            nc.gpsimd.dma_start(wwT4[:], wwT4_d[:])
            # chunks 0+1 back-to-back on the sync queue (they gate the
            # first two exps); the rest striped by need-time
            xqs = [nc.sync, nc.sync, nc.gpsimd, nc.scalar, nc.gpsimd,
                   nc.scalar, nc.sync]
            for k, (off, cw) in enumerate(S_CHUNKS):
                xqs[k].dma_start(x_aug[:, off:off + cw],
                                 x_aug_d[:, off:off + cw])

            with tc.tile_pool(name="ypsum", bufs=1, space="PSUM") as ypool:
                # ---- y accumulators: 13 chunks in 2 x 2-bank tiles ----
                y_ps = [ypool.tile([128, 1024], F32, tag=f"y{t}",
                                   name=f"y{t}") for t in range(2)]

                def y_slot(j):
                    t, s, h = _yslot(j)
                    return y_ps[t][32 * s:32 * s + 32, 512 * h:512 * h + 512]

                # ---- projections (psum borrowed from the y tiles) ----
                def emit_proj_u(k):
                    off, cw = S_CHUNKS[k]
                    pu = y_ps[k % 2][0:2 * CI, 0:cw]
                    for t in range(2):
                        for s2 in range(0, cw, 512):
                            w2 = min(512, cw - s2)
                            nc.tensor.matmul(
                                pu[CI * t:CI * (t + 1), s2:s2 + w2],
                                wuT[:],
                                x_aug[:, off + s2:off + s2 + w2],
                                start=True, stop=True,
                                tile_position=(0, CI * t),
                                skip_group_check=True)
                    if k == 1:
                        # split so block 0's second S matmul starts half
                        # a copy earlier (this copy gates its exp)
                        nc.vector.tensor_copy(u_sb[:, off:off + 512],
                                              pu[:, 0:512])
                        nc.vector.tensor_copy(u_sb[:, off + 512:off + cw],
                                              pu[:, 512:cw])
                    else:
                        nc.vector.tensor_copy(u_sb[:, off:off + cw],
                                              pu[:, :])

                def emit_proj_v(k, sub=None, pv=None):
                    off, cw = V_CHUNKS[k]
                    if sub is not None:
                        off, cw = off + sub[0], sub[1]
                    if pv is None:
                        pv = y_ps[(k + 1) % 2][
                            0:2 * CI,
                            512 + (sub[0] if sub else 0):
                            512 + (sub[0] if sub else 0) + cw]
                    for t in range(2):
                        nc.tensor.matmul(pv[CI * t:CI * (t + 1), :cw], wvT[:],
                                         x_aug[:, off:off + cw],
                                         start=True, stop=True,
                                         tile_position=(0, CI * t),
                                         skip_group_check=True)
                    nc.vector.tensor_copy(v_sb[:, off:off + cw], pv[:, :cw])

                def emit_proj_g(i, pg):
                    nc.tensor.matmul(pg, x_aug[:, i * MB:(i + 1) * MB],
                                     wgT[:], start=True, stop=True,
                                     skip_group_check=True)
                    nc.vector.tensor_copy(gt_sb[:, i * CI:(i + 1) * CI], pg)

                with tc.tile_pool(name="spsum", bufs=2,
                                  space="PSUM") as spool:
                    mm_state = [0]

                    def s_mms(i, sp, off, cw):
                        for s2 in range(0, cw, 512):
                            w2 = min(512, cw - s2)
                            g = CI * (mm_state[0] % 2)  # alternate row grps
                            mm_state[0] += 1
                            nc.tensor.matmul(
                                sp[:, s2:s2 + w2],
                                v_sb[g:g + CI, i * MB:(i + 1) * MB],
                                u_sb[g:g + CI, off + s2:off + s2 + w2],
                                start=True, stop=True)

                    def emit_dve_tail(i, exp_t):
                        # row sums of exp via a DVE pairwise bf16 add tree
                        # (2x_1p perf mode) + 1x partial reduces arranged
                        # so only the last pair's reduce trails the final
                        # exp.  ACT-side accum_out costs a serial
                        # READ_ACCUMULATOR per chunk; DVE-side
                        # tensor_scalar+accum lowers to the 1x
                        # TENSOR_SCALAR_CACHE_REDUCE (6.8us/block).
                        t1 = wpool.tile([128, SCH], BF16, tag="t1",
                                        name="t1")
                        t2 = wpool.tile([128, SCH], BF16, tag="t2",
                                        name="t2")
                        t3 = wpool.tile([128, SCH], BF16, tag="t3",
                                        name="t3")
                        r12 = wpool.tile([128, 1], F32, tag="r12",
                                         name="r12")
                        r3 = wpool.tile([128, 1], F32, tag="r3", name="r3")
                        r6 = wpool.tile([128, 1], F32, tag="r6", name="r6")
                        nc.vector.tensor_reduce(r6[:], exp_t[:, 0:256],
                                                mybir.AxisListType.X, ADD)
                        nc.vector.tensor_tensor(
                            t1[:], exp_t[:, 256:256 + SCH],
                            exp_t[:, 256 + SCH:256 + 2 * SCH], ADD)
                        nc.vector.tensor_tensor(
                            t2[:], exp_t[:, 256 + 2 * SCH:256 + 3 * SCH],
                            exp_t[:, 256 + 3 * SCH:256 + 4 * SCH], ADD)
                        nc.vector.tensor_tensor(t1[:], t1[:], t2[:], ADD)
                        nc.vector.tensor_reduce(r12[:], t1[:],
                                                mybir.AxisListType.X, ADD)
                        tot = wpool.tile([128, 1], F32, tag="tot",
                                         name="tot")
                        if i == NBLK - 1:
                            # last block: reduce the two final chunks
                            # separately so only one [128,1024] 1x reduce
                            # trails the very last exp
                            rE = wpool.tile([128, 1], F32, tag="rE",
                                            name="rE")
                            nc.vector.tensor_reduce(
                                rE[:], exp_t[:, 256 + 4 * SCH:256 + 5 * SCH],
                                mybir.AxisListType.X, ADD)
                            nc.vector.tensor_tensor(tot[:], r12[:], r6[:],
                                                    ADD)
                            nc.vector.tensor_reduce(
                                r3[:], exp_t[:, 256 + 5 * SCH:N],
                                mybir.AxisListType.X, ADD)
                            nc.vector.tensor_tensor(tot[:], tot[:], rE[:],
                                                    ADD)
                            nc.vector.tensor_tensor(tot[:], tot[:], r3[:],
                                                    ADD)
                        else:
                            nc.vector.tensor_tensor(
                                t3[:], exp_t[:, 256 + 4 * SCH:256 + 5 * SCH],
                                exp_t[:, 256 + 5 * SCH:N], ADD)
                            nc.vector.tensor_reduce(r3[:], t3[:],
                                                    mybir.AxisListType.X,
                                                    ADD)
                            nc.vector.tensor_tensor(tot[:], r12[:], r6[:],
                                                    ADD)
                            nc.vector.tensor_tensor(tot[:], tot[:], r3[:],
                                                    ADD)
                        rec = wpool.tile([128, 1], F32, tag="rec",
                                         name="rec")
                        nc.vector.reciprocal(rec[:], tot[:])
                        gts = wpool.tile([128, CI], BF16, tag="gts",
                                         name="gts")
                        nc.vector.tensor_scalar_mul(
                            gts[:], gt_sb[:, i * CI:(i + 1) * CI], rec[:])
                        return gts

                    # block 0: projections interleaved with its S chunks
                    # so the first exp fires as early as possible
                    exp0 = dpool.tile([128, N], BF16, tag="expS",
                                      name="exp_t")
                    # block 0's stationary v (cols 0-128) first, then the
                    # rest of v chunk 0, so the first S matmul fires early
                    emit_proj_v(0, sub=(0, 128))
                    emit_proj_v(0, sub=(128, 384))
                    for ci, (off, cw) in enumerate(S_CHUNKS):
                        emit_proj_u(ci)
                        sp = spool.tile([128, SCH], F32, tag="s", name="sp")
                        s_mms(0, sp, off, cw)
                        nc.scalar.activation(exp0[:, off:off + cw],
                                             sp[:, :cw], EXP)
                    # v1 covers blocks 4-7; v2-v6 ride along in early
                    # iterations so block 0 isn't PE-bound.  Only g0/g1 up
                    # front (borrowing y-tile corners, which y's
                    # start=True matmuls later overwrite); the rest ride
                    # along one per block so the g matmuls never form a
                    # PE-queue convoy.
                    emit_proj_v(1)
                    for i in (0, 1):
                        emit_proj_g(i, y_ps[i][:, 0:32])
                    gts_prev = emit_dve_tail(0, exp0)[:]
                    exp_prev = exp0

                    for i in range(1, NBLK):
                        exp_t = dpool.tile([128, N], BF16, tag="expS",
                                           name="exp_t")
                        for ci, (off, cw) in enumerate(S_CHUNKS):
                            sp = spool.tile([128, SCH], F32, tag="s",
                                            name="sp")
                            s_mms(i, sp, off, cw)
                            nc.scalar.activation(exp_t[:, off:off + cw],
                                                 sp[:, :cw], EXP)
                            if ci == 0 and i + 1 < NBLK:
                                # next block's g projection borrows bank 2
                                # of the 256-wide chunk's tile — keeps the
                                # spool allocation count odd so block
                                # boundaries stay pipelined, and never
                                # touches the accumulating y tiles
                                emit_proj_g(i + 1, sp[:, 512:544])
                            if ci == 0 and 1 <= i <= 5:
                                offv, cwv = V_CHUNKS[i + 1]
                                for sv in range(0, cwv, 256):
                                    wv = min(256, cwv - sv)
                                    emit_proj_v(i + 1, sub=(sv, wv),
                                                pv=sp[0:2 * CI, 768:768 + wv])

                        # y matmuls for the previous block (emitted after
                        # this block's S matmuls so ACT never starves)
                        for j, (off, cw) in enumerate(Y_CHUNKS):
                            _, s, _ = _yslot(j)
                            nc.tensor.matmul(
                                y_slot(j)[:, :cw], gts_prev,
                                exp_prev[:, off:off + cw],
                                start=(i - 1 == 0), stop=(i - 1 == NBLK - 1),
                                tile_position=(0, 32 * s),
                                skip_group_check=True)


                        gts_prev = emit_dve_tail(i, exp_t)[:]
                        exp_prev = exp_t

                    # last block's y matmuls
                    i = NBLK - 1
                    for j, (off, cw) in enumerate(Y_CHUNKS):
                        _, s, _ = _yslot(j)
                        nc.tensor.matmul(
                            y_slot(j)[:, :cw], gts_prev,
                            exp_prev[:, off:off + cw],
                            start=(i == 0), stop=True,
                            tile_position=(0, 32 * s),
                            skip_group_check=True)

                # ---- y psum -> fp16 (then ypool closes, freeing banks)
                nc.scalar.copy(ysA[:, 0:512], y_ps[0][:, 0:512])
                nc.scalar.copy(ysA[:, 512:1024], y_ps[0][:, 512:1024])
                nc.vector.tensor_copy(ysB[:, 0:512], y_ps[1][:, 0:512])
                nc.vector.tensor_copy(ysB[0:32, 512:768],
                                      y_ps[1][0:32, 512:768])

            # ---- epilogue: ww projection and store, 4-deep pipeline ----
            with (
                tc.tile_pool(name="fpsum", bufs=4, space="PSUM") as fpool,
                tc.tile_pool(name="osb", bufs=6) as opool,
            ):
                outqs = [nc.sync, nc.gpsimd]
                dq = 0
                for gp in range(7):
                    j0 = 2 * gp
                    wid = 1024 if gp < 6 else 256
                    fp = fpool.tile([C, 1024], F32, tag="f")
                    for j in (j0, j0 + 1):
                        if j > 12:
                            break
                        off, cw = Y_CHUNKS[j]
                        _, s, hcol = _yslot(j)
                        ys = (ysA if j < 8 else ysB)[
                            32 * s:32 * s + 32,
                            512 * hcol:512 * hcol + cw]
                        nc.tensor.matmul(fp[:, 512 * (j - j0):
                                            512 * (j - j0) + cw],
                                         wwT4[32 * s:32 * s + 32, :], ys,
                                         start=True, stop=True,
                                         tile_position=(32 * s, 0))
                    ot = opool.tile([C, 1024], F16, tag="ot")
                    if gp % 2 == 0:
                        nc.vector.tensor_copy(ot[:, :wid], fp[:, :wid])
                    else:
                        nc.scalar.copy(ot[:, :wid], fp[:, :wid])
                    outqs[dq % 2].dma_start(
                        out_d[:, 1024 * gp:1024 * gp + wid], ot[:, :wid])
                    dq += 1

    nc.compile()
    return nc


def make_in_maps(x, Wg, bg, Wu, bu, Wv, bv, Ww, bw):
    x16 = np.asarray(x, np.float32).astype(np.float16)
    ones = np.ones((1, N), np.float16)
    wuT = np.concatenate([np.asarray(Wu, np.float32).T,
                          np.asarray(bu, np.float32)[None, :]], 0).astype(np.float16)
    wvT = np.concatenate([np.asarray(Wv, np.float32).T,
                          np.asarray(bv, np.float32)[None, :]], 0).astype(np.float16)
    wgT = np.concatenate([np.asarray(Wg, np.float32).T,
                          np.asarray(bg, np.float32)[None, :]], 0).astype(np.float16)
    wwT4 = np.concatenate(
        [np.ascontiguousarray(np.asarray(Ww, np.float32).T)] * 4, 0).astype(np.float16)

    in_maps = []
    for core in range(NCORES):
        b, h = divmod(core, 2)
        xb16 = x16[b].reshape(C, N)
        if h == 1:
            # rotate columns so this core's v/g region sits at cols 0-3200
            xb16 = np.concatenate([xb16[:, MH:], xb16[:, :MH]], axis=1)
        x_aug = np.concatenate([xb16, ones], 0)
        in_maps.append({
            "x_aug": np.ascontiguousarray(x_aug),
            "wuT": np.ascontiguousarray(wuT),
            "wvT": np.ascontiguousarray(wvT),
            "wgT": np.ascontiguousarray(wgT),
            "wwT4": np.ascontiguousarray(wwT4),
        })
    return in_maps


_NC = None


def kernel(x, Wg, bg, Wu, bu, Wv, bv, Ww, bw, _trace=False):
    global _NC
    if _NC is None:
        _NC = build_nc()
    in_maps = make_in_maps(x, Wg, bg, Wu, bu, Wv, bv, Ww, bw)
    res = run_bass_kernel_spmd(_NC, in_maps, list(range(NCORES)), trace=_trace)
    outs = [np.asarray(r["out"], np.float32) for r in res.results]
    x = np.asarray(x, np.float32)
    bw = np.asarray(bw, np.float32)
    full = np.empty((B, C, H, W), np.float32)
    for b in range(B):
        wy = outs[2 * b] + np.roll(outs[2 * b + 1], MH, axis=1)
        full[b] = (wy + bw[:, None] + x[b].reshape(C, N)).reshape(C, H, W)
    kernel.last_results = res
    return full


if __name__ == "__main__":
    rng = np.random.default_rng(0)
    s_in, s_mid = 1.0 / np.sqrt(C), 1.0 / np.sqrt(CI)
    ins = dict(
        x=rng.standard_normal((B, C, H, W), np.float32),
        Wg=(rng.standard_normal((CI, C)) * s_in).astype(np.float32),
        bg=(rng.standard_normal(CI) * 0.01).astype(np.float32),
        Wu=(rng.standard_normal((CI, C)) * s_in).astype(np.float32),
        bu=(rng.standard_normal(CI) * 0.01).astype(np.float32),
        Wv=(rng.standard_normal((CI, C)) * s_in).astype(np.float32),
        bv=(rng.standard_normal(CI) * 0.01).astype(np.float32),
        Ww=(rng.standard_normal((C, CI)) * s_mid).astype(np.float32),
        bw=(rng.standard_normal(C) * 0.01).astype(np.float32),
    )
    out = kernel(**ins)
    print("kernel output", out.shape, out.dtype)


# revision 38
# speedup vs baseline: 1.0231x; 1.0231x over previous
# Trainium2 Bass kernel for NonLocalBlock (B=4, C=64, CI=32, H=W=80).
#
# Math (per batch, N = H*W = 6400):
#   u = Wu@x+bu, v = Wv@x+bv, g = Wg@x+bg           [CI, N]
#   f[n,m] = sum_c u[c,n] v[c,m]; softmax over n (axis=1 of f)
#   y[c,n] = sum_m f_sm[n,m] g[c,m];  out = Ww@y + bw + x
#
# Define S = v^T u  (S[m,n] = f[n,m]).  The softmax axis n is the FREE
# axis of S rows, so 128-row blocks of S make the softmax row-local.
# y = g @ softmax_rows(S).
#
# Sharding: 8 cores = 4 batches x 2 halves of the m axis.  Each core
# computes a partial y over its 3200 m rows and applies the output
# projection; the host sums the halves and adds bias + residual.  The
# odd core's x_aug is column-rotated by 3200 so the same SPMD program
# reads its v/g region at columns 0-3200; the host un-rotates its
# output (a pure relabeling of the n axis — softmax rows are invariant).
#
# Engine budget per core: ACT does only the 175 exp ACTIVATEs
# (~7.2us/block, the bottleneck).  Row sums come from a DVE pairwise
# bf16 add tree (2x_1p mode) + one small 1x reduce (~4.4us/block);
# ACT-side accum_out costs a serial READ_ACCUMULATOR per chunk and
# DVE-side tensor_scalar+accum lowers to the 1x CACHE_REDUCE — both
# measurably slower.  All PSUM->SBUF copies live off the ACT queue.
import numpy as np

import concourse.bass as bass
import concourse.mybir as mybir
from concourse import bacc, tile
from concourse.bass_utils import run_bass_kernel_spmd

F32 = mybir.dt.float32
BF16 = mybir.dt.bfloat16
F16 = mybir.dt.float16

B, C, CI, H, W = 4, 64, 32, 80, 80
N = H * W              # 6400
NCORES = 8
MH = N // 2            # 3200 rows of S per core
MB = 128               # S row-block
NBLK = MH // MB        # 25 blocks per core
SCH = 1024             # S free-dim chunk held in PSUM (2 banks)

EXP = mybir.ActivationFunctionType.Exp
ADD = mybir.AluOpType.add


def _ceil_chunks(total, step):
    out = []
    off = 0
    while off < total:
        out.append((off, min(step, total - off)))
        off += step
    return out


# 256-wide chunk FIRST: the next block's first S matmul then reuses a
# buffer whose last reader is the second-to-last exp of this block, so
# the PE builds it while ACT finishes the block (no boundary stall).
S_CHUNKS = [(0, 256)] + [(256 + SCH * k, SCH) for k in range(6)]
Y_CHUNKS = _ceil_chunks(N, 512)      # 12 x 512 + 256
V_CHUNKS = _ceil_chunks(MH, 512)     # 6 x 512 + 128


def _yslot(j):
    """y chunk j -> (tile index 0/1, partition slot, col half)."""
    if j < 8:
        return 0, j % 4, j // 4
    return 1, (j - 8) % 4, (j - 8) // 4


def build_nc():
    nc = bacc.Bacc("TRN2", target_bir_lowering=False, debug=False,
                   num_devices=NCORES)

    x_aug_d = nc.dram_tensor("x_aug", [C + 1, N], F16, kind="ExternalInput")
    wuT_d = nc.dram_tensor("wuT", [C + 1, CI], F16, kind="ExternalInput")
    wvT_d = nc.dram_tensor("wvT", [C + 1, CI], F16, kind="ExternalInput")
    wgT_d = nc.dram_tensor("wgT", [C + 1, CI], F16, kind="ExternalInput")
    wwT4_d = nc.dram_tensor("wwT4", [128, C], F16, kind="ExternalInput")
    out_d = nc.dram_tensor("out", [C, N], F16, kind="ExternalOutput")

    with tile.TileContext(nc) as tc:
        with (
            tc.tile_pool(name="const", bufs=1) as cpool,
            tc.tile_pool(name="big", bufs=2) as dpool,
            tc.tile_pool(name="small", bufs=3) as wpool,
        ):
            # ---- persistent SBUF tiles ----
            x_aug = cpool.tile([C + 1, N], F16, tag="xa")
            u_sb = cpool.tile([2 * CI, N], F16, tag="u")     # 2 row groups
            v_sb = cpool.tile([2 * CI, MH], F16, tag="v")
            gt_sb = cpool.tile([128, NBLK * CI], F32, tag="gt")
            wuT = cpool.tile([C + 1, CI], F16, tag="wu")
            wvT = cpool.tile([C + 1, CI], F16, tag="wv")
            wgT = cpool.tile([C + 1, CI], F16, tag="wg")
            wwT4 = cpool.tile([128, C], F16, tag="ww")
            ysA = cpool.tile([128, 1024], F16, tag="ysA")
            ysB = cpool.tile([128, 768], F16, tag="ysB")

            # ---- input DMAs: weights first, x_aug striped over queues ----
            nc.gpsimd.dma_start(wuT[:], wuT_d[:])
            nc.scalar.dma_start(wvT[:], wvT_d[:])
            nc.gpsimd.dma_start(wgT[:], wgT_d[:])
            nc.gpsimd.dma_start(wwT4[:], wwT4_d[:])
            # chunks 0+1 back-to-back on the sync queue (they gate the
            # first two exps); the rest striped by need-time
            xqs = [nc.sync, nc.sync, nc.gpsimd, nc.scalar, nc.gpsimd,
                   nc.scalar, nc.sync]
            for k, (off, cw) in enumerate(S_CHUNKS):
                xqs[k].dma_start(x_aug[:, off:off + cw],
                                 x_aug_d[:, off:off + cw])

            with tc.tile_pool(name="ypsum", bufs=1, space="PSUM") as ypool:
                # ---- y accumulators: 13 chunks in 2 x 2-bank tiles ----
                y_ps = [ypool.tile([128, 1024], F32, tag=f"y{t}",
                                   name=f"y{t}") for t in range(2)]

                def y_slot(j):
                    t, s, h = _yslot(j)
                    return y_ps[t][32 * s:32 * s + 32, 512 * h:512 * h + 512]

                # ---- projections (psum borrowed from the y tiles) ----
                def emit_proj_u(k):
                    off, cw = S_CHUNKS[k]
                    pu = y_ps[k % 2][0:2 * CI, 0:cw]
                    for t in range(2):
                        for s2 in range(0, cw, 512):
                            w2 = min(512, cw - s2)
                            nc.tensor.matmul(
                                pu[CI * t:CI * (t + 1), s2:s2 + w2],
                                wuT[:],
                                x_aug[:, off + s2:off + s2 + w2],
                                start=True, stop=True,
                                tile_position=(0, CI * t),
                                skip_group_check=True)
                    if k == 1:
                        # split so block 0's second S matmul starts half
                        # a copy earlier (this copy gates its exp)
                        nc.vector.tensor_copy(u_sb[:, off:off + 512],
                                              pu[:, 0:512])
                        nc.vector.tensor_copy(u_sb[:, off + 512:off + cw],
                                              pu[:, 512:cw])
                    else:
                        nc.vector.tensor_copy(u_sb[:, off:off + cw],
                                              pu[:, :])

                def emit_proj_v(k, sub=None, pv=None):
                    off, cw = V_CHUNKS[k]
                    if sub is not None:
                        off, cw = off + sub[0], sub[1]
                    if pv is None:
                        pv = y_ps[(k + 1) % 2][
                            0:2 * CI,
                            512 + (sub[0] if sub else 0):
                            512 + (sub[0] if sub else 0) + cw]
                    for t in range(2):
                        nc.tensor.matmul(pv[CI * t:CI * (t + 1), :cw], wvT[:],
                                         x_aug[:, off:off + cw],
                                         start=True, stop=True,
                                         tile_position=(0, CI * t),
                                         skip_group_check=True)
                    nc.vector.tensor_copy(v_sb[:, off:off + cw], pv[:, :cw])

                def emit_proj_g(i, pg):
                    nc.tensor.matmul(pg, x_aug[:, i * MB:(i + 1) * MB],
                                     wgT[:], start=True, stop=True,
                                     skip_group_check=True)
                    nc.vector.tensor_copy(gt_sb[:, i * CI:(i + 1) * CI], pg)

                with tc.tile_pool(name="spsum", bufs=2,
                                  space="PSUM") as spool:
                    mm_state = [0]

                    def s_mms(i, sp, off, cw):
                        for s2 in range(0, cw, 512):
                            w2 = min(512, cw - s2)
                            g = CI * (mm_state[0] % 2)  # alternate row grps
                            mm_state[0] += 1
                            nc.tensor.matmul(
                                sp[:, s2:s2 + w2],
                                v_sb[g:g + CI, i * MB:(i + 1) * MB],
                                u_sb[g:g + CI, off + s2:off + s2 + w2],
                                start=True, stop=True)

                    def emit_dve_tail(i, exp_t):
                        # row sums of exp via a DVE pairwise bf16 add tree
                        # (2x_1p perf mode) + 1x partial reduces arranged
                        # so only the last pair's reduce trails the final
                        # exp.  ACT-side accum_out costs a serial
                        # READ_ACCUMULATOR per chunk; DVE-side
                        # tensor_scalar+accum lowers to the 1x
                        # TENSOR_SCALAR_CACHE_REDUCE (6.8us/block).
                        t1 = wpool.tile([128, SCH], BF16, tag="t1",
                                        name="t1")
                        t2 = wpool.tile([128, SCH], BF16, tag="t2",
                                        name="t2")
                        t3 = wpool.tile([128, SCH], BF16, tag="t3",
                                        name="t3")
                        r12 = wpool.tile([128, 1], F32, tag="r12",
                                         name="r12")
                        r3 = wpool.tile([128, 1], F32, tag="r3", name="r3")
                        r6 = wpool.tile([128, 1], F32, tag="r6", name="r6")
                        nc.vector.tensor_reduce(r6[:], exp_t[:, 0:256],
                                                mybir.AxisListType.X, ADD)
                        nc.vector.tensor_tensor(
                            t1[:], exp_t[:, 256:256 + SCH],
                            exp_t[:, 256 + SCH:256 + 2 * SCH], ADD)
                        nc.vector.tensor_tensor(
                            t2[:], exp_t[:, 256 + 2 * SCH:256 + 3 * SCH],
                            exp_t[:, 256 + 3 * SCH:256 + 4 * SCH], ADD)
                        nc.vector.tensor_tensor(t1[:], t1[:], t2[:], ADD)
                        nc.vector.tensor_reduce(r12[:], t1[:],
                                                mybir.AxisListType.X, ADD)
                        tot = wpool.tile([128, 1], F32, tag="tot",
                                         name="tot")
                        if i == NBLK - 1:
                            # last block: reduce the two final chunks
                            # separately so only one [128,1024] 1x reduce
                            # trails the very last exp
                            rE = wpool.tile([128, 1], F32, tag="rE",
                                            name="rE")
                            nc.vector.tensor_reduce(
                                rE[:], exp_t[:, 256 + 4 * SCH:256 + 5 * SCH],
                                mybir.AxisListType.X, ADD)
                            nc.vector.tensor_tensor(tot[:], r12[:], r6[:],
                                                    ADD)
                            nc.vector.tensor_reduce(
                                r3[:], exp_t[:, 256 + 5 * SCH:N],
                                mybir.AxisListType.X, ADD)
                            nc.vector.tensor_tensor(tot[:], tot[:], rE[:],
                                                    ADD)
                            nc.vector.tensor_tensor(tot[:], tot[:], r3[:],
                                                    ADD)
                        else:
                            nc.vector.tensor_tensor(
                                t3[:], exp_t[:, 256 + 4 * SCH:256 + 5 * SCH],
                                exp_t[:, 256 + 5 * SCH:N], ADD)
                            nc.vector.tensor_reduce(r3[:], t3[:],
                                                    mybir.AxisListType.X,
                                                    ADD)
                            nc.vector.tensor_tensor(tot[:], r12[:], r6[:],
                                                    ADD)
                            nc.vector.tensor_tensor(tot[:], tot[:], r3[:],
                                                    ADD)
                        rec = wpool.tile([128, 1], F32, tag="rec",
                                         name="rec")
                        nc.vector.reciprocal(rec[:], tot[:])
                        gts = wpool.tile([128, CI], BF16, tag="gts",
                                         name="gts")
                        nc.vector.tensor_scalar_mul(
                            gts[:], gt_sb[:, i * CI:(i + 1) * CI], rec[:])
                        return gts

                    # block 0: projections interleaved with its S chunks
                    # so the first exp fires as early as possible
                    exp0 = dpool.tile([128, N], BF16, tag="expS",
                                      name="exp_t")
                    # u chunk 0 first (wuT lands before wvT), then just
                    # block 0's 128-col stationary v — the shortest chain
                    # to the first exp.  v cols 128+ feed later blocks
                    # and ride after the block-0 S chunks.
                    emit_proj_u(0)
                    emit_proj_v(0, sub=(0, 128))
                    for ci, (off, cw) in enumerate(S_CHUNKS):
                        if ci > 0:
                            emit_proj_u(ci)
                        sp = spool.tile([128, SCH], F32, tag="s", name="sp")
                        s_mms(0, sp, off, cw)
                        nc.scalar.activation(exp0[:, off:off + cw],
                                             sp[:, :cw], EXP)
                    # rest of v; only g0/g1 up front (borrowing y-tile
                    # corners, which y's start=True matmuls later
                    # overwrite) — the other g projections ride along one
                    # per block so they never form a PE-queue convoy
                    emit_proj_v(0, sub=(128, 384))
                    for k in range(1, len(V_CHUNKS)):
                        emit_proj_v(k)
                    for i in (0, 1):
                        emit_proj_g(i, y_ps[i][:, 0:32])
                    gts_prev = emit_dve_tail(0, exp0)[:]
                    exp_prev = exp0

                    for i in range(1, NBLK):
                        exp_t = dpool.tile([128, N], BF16, tag="expS",
                                           name="exp_t")
                        for ci, (off, cw) in enumerate(S_CHUNKS):
                            sp = spool.tile([128, SCH], F32, tag="s",
                                            name="sp")
                            s_mms(i, sp, off, cw)
                            nc.scalar.activation(exp_t[:, off:off + cw],
                                                 sp[:, :cw], EXP)
                            if ci == 0 and i + 1 < NBLK:
                                # next block's g projection borrows bank 2
                                # of the 256-wide chunk's tile — keeps the
                                # spool allocation count odd so block
                                # boundaries stay pipelined, and never
                                # touches the accumulating y tiles
                                emit_proj_g(i + 1, sp[:, 512:544])


                        # y matmuls for the previous block (emitted after
                        # this block's S matmuls so ACT never starves)
                        for j, (off, cw) in enumerate(Y_CHUNKS):
                            _, s, _ = _yslot(j)
                            nc.tensor.matmul(
                                y_slot(j)[:, :cw], gts_prev,
                                exp_prev[:, off:off + cw],
                                start=(i - 1 == 0), stop=(i - 1 == NBLK - 1),
                                tile_position=(0, 32 * s),
                                skip_group_check=True)


                        gts_prev = emit_dve_tail(i, exp_t)[:]
                        exp_prev = exp_t

                    # last block's y matmuls
                    i = NBLK - 1
                    for j, (off, cw) in enumerate(Y_CHUNKS):
                        _, s, _ = _yslot(j)
                        nc.tensor.matmul(
                            y_slot(j)[:, :cw], gts_prev,
                            exp_prev[:, off:off + cw],
                            start=(i == 0), stop=True,
                            tile_position=(0, 32 * s),
                            skip_group_check=True)

                # ---- y psum -> fp16 (then ypool closes, freeing banks)
                nc.scalar.copy(ysA[:, 0:512], y_ps[0][:, 0:512])
                nc.scalar.copy(ysA[:, 512:1024], y_ps[0][:, 512:1024])
                nc.vector.tensor_copy(ysB[:, 0:512], y_ps[1][:, 0:512])
                nc.vector.tensor_copy(ysB[0:32, 512:768],
                                      y_ps[1][0:32, 512:768])

            # ---- epilogue: ww projection and store, 4-deep pipeline ----
            with (
                tc.tile_pool(name="fpsum", bufs=4, space="PSUM") as fpool,
                tc.tile_pool(name="osb", bufs=6) as opool,
            ):
                outqs = [nc.sync, nc.gpsimd]
                dq = 0
                for gp in range(7):
                    j0 = 2 * gp
                    wid = 1024 if gp < 6 else 256
                    fp = fpool.tile([C, 1024], F32, tag="f")
                    for j in (j0, j0 + 1):
                        if j > 12:
                            break
                        off, cw = Y_CHUNKS[j]
                        _, s, hcol = _yslot(j)
                        ys = (ysA if j < 8 else ysB)[
                            32 * s:32 * s + 32,
                            512 * hcol:512 * hcol + cw]
                        nc.tensor.matmul(fp[:, 512 * (j - j0):
                                            512 * (j - j0) + cw],
                                         wwT4[32 * s:32 * s + 32, :], ys,
                                         start=True, stop=True,
                                         tile_position=(32 * s, 0))
                    ot = opool.tile([C, 1024], F16, tag="ot")
                    if gp % 2 == 0:
                        nc.vector.tensor_copy(ot[:, :wid], fp[:, :wid])
                    else:
                        nc.scalar.copy(ot[:, :wid], fp[:, :wid])
                    outqs[dq % 2].dma_start(
                        out_d[:, 1024 * gp:1024 * gp + wid], ot[:, :wid])
                    dq += 1

    nc.compile()
    return nc


def make_in_maps(x, Wg, bg, Wu, bu, Wv, bv, Ww, bw):
    x16 = np.asarray(x, np.float32).astype(np.float16)
    ones = np.ones((1, N), np.float16)
    wuT = np.concatenate([np.asarray(Wu, np.float32).T,
                          np.asarray(bu, np.float32)[None, :]], 0).astype(np.float16)
    wvT = np.concatenate([np.asarray(Wv, np.float32).T,
                          np.asarray(bv, np.float32)[None, :]], 0).astype(np.float16)
    wgT = np.concatenate([np.asarray(Wg, np.float32).T,
                          np.asarray(bg, np.float32)[None, :]], 0).astype(np.float16)
    wwT4 = np.concatenate(
        [np.ascontiguousarray(np.asarray(Ww, np.float32).T)] * 4, 0).astype(np.float16)

    in_maps = []
    for core in range(NCORES):
        b, h = divmod(core, 2)
        xb16 = x16[b].reshape(C, N)
        if h == 1:
            # rotate columns so this core's v/g region sits at cols 0-3200
            xb16 = np.concatenate([xb16[:, MH:], xb16[:, :MH]], axis=1)
        x_aug = np.concatenate([xb16, ones], 0)
        in_maps.append({
            "x_aug": np.ascontiguousarray(x_aug),
            "wuT": np.ascontiguousarray(wuT),
            "wvT": np.ascontiguousarray(wvT),
            "wgT": np.ascontiguousarray(wgT),
            "wwT4": np.ascontiguousarray(wwT4),
        })
    return in_maps


_NC = None


def kernel(x, Wg, bg, Wu, bu, Wv, bv, Ww, bw, _trace=False):
    global _NC
    if _NC is None:
        _NC = build_nc()
    in_maps = make_in_maps(x, Wg, bg, Wu, bu, Wv, bv, Ww, bw)
    res = run_bass_kernel_spmd(_NC, in_maps, list(range(NCORES)), trace=_trace)
    outs = [np.asarray(r["out"], np.float32) for r in res.results]
    x = np.asarray(x, np.float32)
    bw = np.asarray(bw, np.float32)
    full = np.empty((B, C, H, W), np.float32)
    for b in range(B):
        wy = outs[2 * b] + np.roll(outs[2 * b + 1], MH, axis=1)
        full[b] = (wy + bw[:, None] + x[b].reshape(C, N)).reshape(C, H, W)
    kernel.last_results = res
    return full


if __name__ == "__main__":
    rng = np.random.default_rng(0)
    s_in, s_mid = 1.0 / np.sqrt(C), 1.0 / np.sqrt(CI)
    ins = dict(
        x=rng.standard_normal((B, C, H, W), np.float32),
        Wg=(rng.standard_normal((CI, C)) * s_in).astype(np.float32),
        bg=(rng.standard_normal(CI) * 0.01).astype(np.float32),
        Wu=(rng.standard_normal((CI, C)) * s_in).astype(np.float32),
        bu=(rng.standard_normal(CI) * 0.01).astype(np.float32),
        Wv=(rng.standard_normal((CI, C)) * s_in).astype(np.float32),
        bv=(rng.standard_normal(CI) * 0.01).astype(np.float32),
        Ww=(rng.standard_normal((C, CI)) * s_mid).astype(np.float32),
        bw=(rng.standard_normal(C) * 0.01).astype(np.float32),
    )
    out = kernel(**ins)
    print("kernel output", out.shape, out.dtype)
